# revision 16
# baseline (speedup 1.0000x reference)
"""Trainium2 Bass kernel for nn_Decoder (latent-grid decoder MLP).

Contract: kernel(**inputs) takes the FULL unsharded inputs (as produced by
setup_inputs()) and returns the FULL [65536, 4] float32 output. Internally the
65536 points are sharded across 8 NeuronCores (pure data parallel); the small
weights are replicated.

Algorithm (mathematically equivalent to the reference):
  - G=2 trilinear interp of a per-sample 2x2x2 grid always lands in cell
    (0,0,0) (indices clip to [0, G-2] = [0,0]), so
    lat_i = sum_m w_m(xyz) * (lat @ A_m), A_m = convT_w[:, :, di, dj, dk].
  - The interp + Fourier features + first MLP layer fold into one matmul:
    u = [w_0*lat, ..., w_7*lat, sin(2 pi ang), cos(2 pi ang)]  (2304 dims),
    h0 = u @ M0 with M0 = [A_stack @ W0_top; W0_sin; W0_cos] (host-folded).
  - LayerNorm mean-subtraction folds into the weights (column centering);
    ln gamma folds in too. The per-sample rstd is deferred via LN's positive
    scale invariance: activations stay unnormalized, and gi2 (squared inverse
    scale) follows gi2' = ssq_w/512 + eps*gi2, applied once at the end.
    Requires all biases and ln_b == 0 (true for this model; a numpy fallback
    covers the general case).
  - ssq_w and the eps*gi2 term are accumulated by the TensorEngine itself
    (weighted-ones stationary operands producing a broadcast PSUM tile).

Performance structure (v2):
  - Input arranged so each partition reads a contiguous 4144B DRAM segment
    per block DMA (samples mapped s = b*512 + p*4 + sc); the resulting
    within-block sample permutation is undone on the host.
  - Stationary weights in bf16 (FWL-eligible fast weight loads, half DMA);
    moving activations stay fp32r (full PE rate at 512-wide).
  - lat transposed via bf16 PE transposes (1 cyc/row vs 2 for fp32).
  - Software-pipelined emission: block b+1's preamble (input gate, PE
    transposes, corner weights, fourier prep, u-chunk builds) is emitted in
    the middle of block b's hidden layers; block b's epilogue (LN-stats
    matmuls, output layer, finalize) is deferred until after block b+1's
    layer-0 matmuls, so the in-order PE queue never stalls on the ACT-engine
    square/relu chain.
  - PSUM: 6 banks round-robin for the layer accumulators (breaks the
    layer-boundary wait on ACT), 2 banks for all small/aux tiles.
"""

import os
import numpy as np

N_CORES = 8
N_TOTAL = 65536
S_CORE = N_TOTAL // N_CORES          # 8192 samples per core
BLK = 512                            # samples per block
N_BLOCKS = S_CORE // BLK             # 16
EPS = 1e-5
N_LAYERS = 8                         # LN+relu layers (layer0 + 7 hidden)


def _bf16(a):
    import ml_dtypes
    return np.ascontiguousarray(np.asarray(a, np.float32).astype(ml_dtypes.bfloat16))


def _precompute(inputs):
    """Host-side weight folding. Returns dict of constant arrays."""
    convT_w = np.asarray(inputs["convT_w"], np.float64)
    W0 = np.asarray(inputs["W0"], np.float64)
    Wh = np.asarray(inputs["Wh"], np.float64)
    ln_g = np.asarray(inputs["ln_g"], np.float64)
    gauss = np.asarray(inputs["gauss"], np.float32)
    W_out = np.asarray(inputs["W_out"], np.float64)

    # A_stack[m*256+i, c] = convT_w[i, c, di, dj, dk], m = 4*di + 2*dj + dk
    A_stack = convT_w.transpose(2, 3, 4, 0, 1).reshape(8 * 256, 512)
    M0 = np.concatenate([A_stack @ W0[:512], W0[512:640], W0[640:768]], axis=0)

    def center_scale(W, g):
        Wc = W - W.mean(axis=1, keepdims=True)
        return Wc * g[None, :]

    W_eff = [center_scale(M0, ln_g[0])] + [
        center_scale(Wh[l], ln_g[l + 1]) for l in range(7)
    ]
    # pack each layer's weights as [128, n_kchunks, 512]
    def pack(W):
        K = W.shape[0]
        kc = K // 128
        return W.reshape(kc, 128, 512).transpose(1, 0, 2).reshape(128, kc * 512)

    w0p = np.ascontiguousarray(pack(W_eff[0]).astype(np.float32))                                      # [128, 18*512]
    whp = np.ascontiguousarray(np.concatenate([pack(W) for W in W_eff[1:]], axis=1).astype(np.float32))  # [128, 28*512]
    # LN-stat scale: sq_k = (s_k * h_k)^2 with s_k = 1/(sqrt(512)*g_7[k]),
    # summed by a ones-matmul. (The eps*gi2_6 correction term is ~1e-4
    # relative and is dropped.)
    s7 = (1.0 / (np.sqrt(512.0) * np.abs(ln_g[7]))).astype(np.float32)
    sqs = np.ascontiguousarray(s7.reshape(4, 128).T.copy())          # [128, 4]
    ones4 = np.ones((128, 4), np.float32)

    return {
        "w0p": w0p,
        "whp": whp,
        "sqs": sqs,
        "ones4": ones4,
        "identb": _bf16(np.eye(128, dtype=np.float32)),
        "identr": np.ascontiguousarray(np.eye(128, dtype=np.float32)),
        "gaussT": np.ascontiguousarray(np.concatenate(
            [gauss.T.astype(np.float32), np.zeros((8, 128), np.float32)], axis=0)),
        "sel8": np.ascontiguousarray(np.concatenate(
            [np.zeros((3, 8 * 128), np.float32),
             np.kron(np.eye(8, dtype=np.float32), np.ones((1, 128), np.float32))],
            axis=0)),
        "woutp": np.ascontiguousarray(
            np.asarray(W_out, np.float32).reshape(4, 128, 4)
            .transpose(1, 0, 2).reshape(128, 16)),
    }


def _general_case_needed(inputs):
    z = lambda a: bool(np.all(np.asarray(a) == 0))
    return not (
        z(inputs["convT_b"]) and z(inputs["b0"]) and z(inputs["bh"])
        and z(inputs["ln_b"]) and z(inputs["b_out"])
        and bool(np.all(np.abs(np.asarray(inputs["ln_g"])) > 1e-3))
    )


def _numpy_fallback(inputs):
    """Reference in numpy (slow; only for inputs outside the fast path)."""
    inp = np.asarray(inputs["input"], np.float32)
    convT_w = np.asarray(inputs["convT_w"], np.float32)
    convT_b = np.asarray(inputs["convT_b"], np.float32)
    gauss = np.asarray(inputs["gauss"], np.float32)
    W0 = np.asarray(inputs["W0"], np.float32)
    b0 = np.asarray(inputs["b0"], np.float32)
    Wh = np.asarray(inputs["Wh"], np.float32)
    bh = np.asarray(inputs["bh"], np.float32)
    ln_g = np.asarray(inputs["ln_g"], np.float32)
    ln_b = np.asarray(inputs["ln_b"], np.float32)
    W_out = np.asarray(inputs["W_out"], np.float32)
    b_out = np.asarray(inputs["b_out"], np.float32)
    xyz = inp[:, -3:]
    lat = inp[:, :-3]
    f = (xyz + 1.0) * 0.5
    frac = f - np.clip(f.astype(np.int32), 0, 0)
    A = convT_w.transpose(2, 3, 4, 0, 1)
    lat_i = np.zeros((inp.shape[0], 512), np.float32)
    wx = [1 - frac[:, 0], frac[:, 0]]
    wy = [1 - frac[:, 1], frac[:, 1]]
    wz = [1 - frac[:, 2], frac[:, 2]]
    for di in (0, 1):
        for dj in (0, 1):
            for dk in (0, 1):
                w = (wx[di] * wy[dj] * wz[dk]).astype(np.float32)
                lat_i += (lat @ A[di, dj, dk]) * w[:, None]
    lat_i += convT_b[None, :]
    ang = 2 * np.pi * (xyz @ gauss.T)
    x = np.concatenate([lat_i, np.sin(ang), np.cos(ang)], axis=1)

    def ln(t, g, b):
        mu = t.mean(-1, keepdims=True)
        var = ((t - mu) ** 2).mean(-1, keepdims=True)
        return (t - mu) / np.sqrt(var + EPS) * g + b

    x = np.maximum(ln(x @ W0 + b0, ln_g[0], ln_b[0]), 0)
    for l in range(7):
        x = np.maximum(ln(x @ Wh[l] + bh[l], ln_g[l + 1], ln_b[l + 1]), 0)
    y = x @ W_out + b_out
    return np.concatenate([np.tanh(y[:, :1]), y[:, 1:] * 255.0], axis=1).astype(np.float32)


_NC_CACHE = {}


def _build_bass(s_core=S_CORE):
    """Build the per-core Bass module (SPMD; same program on all 8 cores)."""
    import concourse.bass as bass
    import concourse.bacc as bacc
    import concourse.tile as tile
    from concourse import mybir

    FP32 = mybir.dt.float32
    FP32R = mybir.dt.float32r
    BF16 = mybir.dt.bfloat16
    AF = mybir.ActivationFunctionType
    ALU = mybir.AluOpType
    TWO_PI = float(2.0 * np.pi)
    MAGIC = 12582912.0            # 1.5 * 2^23: fp32 add/sub rounds to integer
    n_blocks = s_core // BLK

    nc = bacc.Bacc("TRN2", target_bir_lowering=False, debug=False)

    inp_d = nc.dram_tensor("inp", [s_core, 259], FP32, kind="ExternalInput").ap()
    w0p_d = nc.dram_tensor("w0p", [128, 18 * 512], FP32R, kind="ExternalInput").ap()
    whp_d = nc.dram_tensor("whp", [128, 28 * 512], FP32R, kind="ExternalInput").ap()
    sqs_d = nc.dram_tensor("sqs", [128, 4], FP32, kind="ExternalInput").ap()
    ones4_d = nc.dram_tensor("ones4", [128, 4], FP32R, kind="ExternalInput").ap()
    identb_d = nc.dram_tensor("identb", [128, 128], BF16, kind="ExternalInput").ap()
    identr_d = nc.dram_tensor("identr", [128, 128], FP32R, kind="ExternalInput").ap()
    gaussT_d = nc.dram_tensor("gaussT", [11, 128], FP32R, kind="ExternalInput").ap()
    sel8_d = nc.dram_tensor("sel8", [11, 8 * 128], FP32R, kind="ExternalInput").ap()
    woutp_d = nc.dram_tensor("woutp", [128, 16], FP32R, kind="ExternalInput").ap()
    oraw_d = nc.dram_tensor("oraw", [5, s_core], FP32, kind="ExternalOutput").ap()

    with tile.TileContext(nc) as tc:
        with (
            tc.tile_pool(name="const", bufs=1) as constp,
            tc.tile_pool(name="weights", bufs=1) as weightp,
            tc.tile_pool(name="inblk", bufs=2) as inp_pool,
            tc.tile_pool(name="acts", bufs=2) as actp,
            tc.tile_pool(name="scratch", bufs=2) as scr,
            tc.tile_pool(name="ps_main", bufs=5, space="PSUM") as ps_main,
            tc.tile_pool(name="ps_aux", bufs=2, space="PSUM") as ps_aux,
        ):
            # ---- constants / weights (loaded once, resident) ----
            # Input block DMAs are issued first (below) so block 0's preamble
            # is not stuck behind the weight DMAs on the sync queue.
            # sample s = b*512 + p*4 + sc  ->  [b][p][sc][f], 4144B contiguous
            # per partition per block.
            inp_r = inp_d.rearrange("(b p sc) f -> b p sc f", p=128, sc=4)

            def dma_inb(b):
                t = inp_pool.tile([128, 4, 259], FP32, tag="inb0", name="inb0")
                nc.sync.dma_start(out=t, in_=inp_r[b])
                return t

            inb0_tiles = {}
            inb0_tiles[0] = dma_inb(0)
            inb0_tiles[1] = dma_inb(1)

            identb_dma = constp.tile([128, 128], BF16, name="identb_dma")
            nc.sync.dma_start(out=identb_dma, in_=identb_d)
            identb_sb = constp.tile([128, 128], BF16, name="identb_sb")
            nc.vector.tensor_copy(identb_sb, identb_dma)
            identr_dma = constp.tile([128, 128], FP32R, name="identr_dma")
            nc.sync.dma_start(out=identr_dma, in_=identr_d)
            identr_sb = constp.tile([128, 128], FP32R, name="identr_sb")
            nc.vector.tensor_copy(identr_sb, identr_dma)
            gaussT_sb = constp.tile([11, 128], FP32R)
            nc.sync.dma_start(out=gaussT_sb, in_=gaussT_d)
            sel8_sb = constp.tile([11, 8, 128], FP32R)
            nc.sync.dma_start(out=sel8_sb, in_=sel8_d.rearrange("p (m f) -> p m f", m=8))
            wout_sb = weightp.tile([128, 4, 4], FP32R)
            nc.sync.dma_start(out=wout_sb, in_=woutp_d.rearrange("p (c f) -> p c f", c=4))
            sqs_sb = weightp.tile([128, 4], FP32)
            nc.sync.dma_start(out=sqs_sb, in_=sqs_d)
            ones4_sb = weightp.tile([128, 4], FP32R)
            nc.sync.dma_start(out=ones4_sb, in_=ones4_d)

            w0_sb = weightp.tile([128, 18, 512], FP32R)
            w0r = w0p_d.rearrange("p (c f) -> p c f", c=18)
            for ch in range(3):
                nc.sync.dma_start(
                    out=w0_sb[:, ch * 6:(ch + 1) * 6, :], in_=w0r[:, ch * 6:(ch + 1) * 6, :])
            wh_sb = weightp.tile([128, 28, 512], FP32R)
            whr = whp_d.rearrange("p (c f) -> p c f", c=28)
            for ch in range(4):
                nc.sync.dma_start(
                    out=wh_sb[:, ch * 7:(ch + 1) * 7, :], in_=whr[:, ch * 7:(ch + 1) * 7, :])

            # ---------------- pipelined block structure ----------------

            def preamble_a0(b):
                """Input gate + corner-weight build (DVE only) for block b.
                Emitted during block b-1's layer 1."""
                h = {}
                inb0 = inb0_tiles.pop(b)
                # gate: lat -> bf16 (feeds bf16 PE transposes); xyz -> comb
                latb = inp_pool.tile([128, 4, 256], BF16, tag="latb", name="latb",
                                     bufs=1)
                for sc in range(4):
                    nc.vector.tensor_copy(latb[:, sc, :], inb0[:, sc, 0:256])
                # comb[:, :, 0:3] = xyz, [:, :, 3:11] = corner weights w8
                comb = scr.tile([128, 4, 11], FP32R, tag="comb")
                nc.vector.tensor_copy(comb[:, :, 0:3], inb0[:, :, 256:259])
                xyzf = comb[:, :, 0:3]

                # corner-weight chain (sample-major)
                f3 = scr.tile([128, 4, 3], FP32R, tag="f3")
                nc.vector.tensor_scalar(
                    out=f3, in0=xyzf, scalar1=0.5, scalar2=0.5,
                    op0=ALU.mult, op1=ALU.add,
                )
                om3 = scr.tile([128, 4, 3], FP32R, tag="om3")
                nc.vector.tensor_scalar(
                    out=om3, in0=f3, scalar1=1.0, scalar2=-1.0,
                    op0=ALU.subtract, op1=ALU.mult,
                )
                wxy = scr.tile([128, 4, 4], FP32R, tag="wxy")
                for di in (0, 1):
                    xs = (f3 if di else om3)[:, :, 0:1]
                    for dj in (0, 1):
                        ys = (f3 if dj else om3)[:, :, 1:2]
                        nc.vector.tensor_tensor(
                            out=wxy[:, :, di * 2 + dj:di * 2 + dj + 1],
                            in0=xs, in1=ys, op=ALU.mult,
                        )
                for m in range(8):
                    di, dj, dk = (m >> 2) & 1, (m >> 1) & 1, m & 1
                    zsl = (f3 if dk else om3)[:, :, 2:3]
                    nc.vector.tensor_tensor(
                        out=comb[:, :, 3 + m:4 + m],
                        in0=wxy[:, :, di * 2 + dj:di * 2 + dj + 1],
                        in1=zsl, op=ALU.mult,
                    )

                h.update(latb=latb, comb=comb)
                return h

            def preamble_a1_groups(b, h):
                """PE transpose groups for block b, returned as closures that
                get interleaved between hidden-layer matmul groups (so the
                PSUM-drain copies never stall the PE). 4 transposes share one
                PSUM tile; one merged drain copy each."""
                latb, comb = h["latb"], h["comb"]
                latT = scr.tile([128, 2, BLK], BF16, tag="latT", bufs=1)
                combT = scr.tile([11, BLK], FP32R, tag="combT", bufs=1)

                def lat_half(half):
                    def go():
                        tpl = ps_aux.tile([128, 4, 128], BF16, tag="tp", bufs=1,
                                          name="tpl")
                        for s2 in range(2):
                            sc = half * 2 + s2
                            for fc in range(2):
                                nc.tensor.transpose(
                                    tpl[:, s2 * 2 + fc, :],
                                    latb[:, sc, fc * 128:(fc + 1) * 128],
                                    identb_sb
                                )
                        dst = latT[:, :, half * 256:(half + 1) * 256].rearrange(
                            "p f (s q) -> p s f q", s=2)
                        nc.vector.tensor_copy(dst, tpl)
                    return go

                def comb_grp():
                    tpc = ps_aux.tile([11, 4, 128], FP32R, tag="tp", bufs=1,
                                      name="tpc")
                    for sc in range(4):
                        nc.tensor.transpose(tpc[:, sc, :], comb[:, sc, :],
                                            identr_sb)
                    nc.vector.tensor_copy(
                        combT.rearrange("p (s q) -> p s q", s=4), tpc)

                h.update(latT=latT, combT=combT)
                return [lat_half(0), comb_grp, lat_half(1)]

            def preamble_b(b, h):
                """Fourier prep + corner broadcasts + u-chunk builds for block
                b. Emitted mid block b-1; the sin/cos ACT ops are emitted later
                (emit_ff) so they don't delay relu ops on the ACT queue."""
                # fourier angle, range-reduced to [-0.5, 0.5]
                angp = ps_aux.tile([128, BLK], FP32, tag="aux")
                nc.tensor.matmul(angp, gaussT_sb, h["combT"], start=True, stop=True)
                ang_sb = scr.tile([128, BLK], FP32, tag="rr", bufs=2, name="ang_sb")
                nc.vector.tensor_copy(ang_sb, angp)
                zs_r = scr.tile([128, BLK], FP32, tag="rr", bufs=2, name="zs_r")
                nc.vector.tensor_scalar(
                    out=zs_r, in0=ang_sb, scalar1=MAGIC, scalar2=MAGIC,
                    op0=ALU.add, op1=ALU.subtract,
                )
                zs = scr.tile([128, BLK], FP32, tag="zs", bufs=1)
                nc.vector.tensor_sub(zs, ang_sb, zs_r)
                a25 = scr.tile([128, BLK], FP32, tag="a25", bufs=1)
                nc.vector.tensor_scalar_add(out=a25, in0=ang_sb, scalar1=0.25)
                zc_r = scr.tile([128, BLK], FP32, tag="rr", bufs=2, name="zc_r")
                nc.vector.tensor_scalar(
                    out=zc_r, in0=a25, scalar1=MAGIC, scalar2=MAGIC,
                    op0=ALU.add, op1=ALU.subtract,
                )
                zc = scr.tile([128, BLK], FP32, tag="zc", bufs=1)
                nc.vector.tensor_sub(zc, a25, zc_r)
                h.update(zs=zs, zc=zc)

                # corner broadcasts + weighted-lat u chunks
                uchs = []
                for m in range(8):
                    bc = ps_aux.tile([128, BLK], FP32, tag="aux")
                    nc.tensor.matmul(
                        bc, sel8_sb[:, m, :], h["combT"], start=True, stop=True
                    )
                    for kc in range(2):
                        uch = scr.tile([128, BLK], FP32R, tag="uch", bufs=18)
                        nc.vector.tensor_tensor(
                            out=uch, in0=h["latT"][:, kc, :], in1=bc, op=ALU.mult
                        )
                        uchs.append(uch)
                h["uchs"] = uchs

            def emit_ff(b, h):
                """sin/cos ACT ops -> u chunks 16/17. Emitted at the start of
                block b's own iteration: the Sin table load and both sins run
                on ACT while the PE streams layer 0 (no relu is urgent)."""
                ffs = scr.tile([128, BLK], FP32R, tag="uch", bufs=18)
                nc.scalar.activation(out=ffs, in_=h["zs"], func=AF.Sin, scale=TWO_PI)
                h["uchs"].append(ffs)
                ffc = scr.tile([128, BLK], FP32R, tag="uch", bufs=18)
                nc.scalar.activation(out=ffc, in_=h["zc"], func=AF.Sin, scale=TWO_PI)
                h["uchs"].append(ffc)

            def layer0(b, h):
                """18-chunk folded first layer for block b."""
                psums = [ps_main.tile([128, BLK], FP32, tag="pt", name=f"pt{i}") for i in range(4)]
                uchs = h["uchs"]
                for mc in range(4):
                    for i, uch in enumerate(uchs):
                        nc.tensor.matmul(
                            psums[mc],
                            w0_sb[:, i, mc * 128:(mc + 1) * 128],
                            uch,
                            start=(i == 0), stop=(i == 17),
                        )
                return psums

            def relu_layer(b, j, psums, want_sq):
                x_next = actp.tile([128, 4, BLK], FP32R, tag="xn")
                for mc in range(4):
                    nc.scalar.activation(
                        out=x_next[:, mc, :], in_=psums[mc], func=AF.Relu
                    )
                sq = None
                if want_sq:
                    # scaled squares: (s_k h_k)^2; chunk-sum on DVE so the
                    # stats need only one ones-matmul
                    sq = scr.tile([128, 4, BLK], FP32R, tag="sq", bufs=2)
                    for mc in range(4):
                        nc.scalar.activation(
                            out=sq[:, mc, :], in_=psums[mc], func=AF.Square,
                            scale=sqs_sb[:, mc:mc + 1],
                        )
                    nc.vector.tensor_tensor(
                        out=sq[:, 0, :], in0=sq[:, 0, :], in1=sq[:, 1, :],
                        op=ALU.add)
                    nc.vector.tensor_tensor(
                        out=sq[:, 2, :], in0=sq[:, 2, :], in1=sq[:, 3, :],
                        op=ALU.add)
                    nc.vector.tensor_tensor(
                        out=sq[:, 0, :], in0=sq[:, 0, :], in1=sq[:, 2, :],
                        op=ALU.add)
                return x_next, sq

            def hidden_mms(b, j, x_cur, interleave=None):
                psums = [ps_main.tile([128, BLK], FP32, tag="pt", name=f"pt{i}") for i in range(4)]
                for mc in range(4):
                    for kc in range(4):
                        nc.tensor.matmul(
                            psums[mc],
                            wh_sb[:, (j - 1) * 4 + kc, mc * 128:(mc + 1) * 128],
                            x_cur[:, kc, :],
                            start=(kc == 0), stop=(kc == 3),
                        )
                    if interleave and mc < len(interleave):
                        interleave[mc]()
                return psums

            def epilogue(b, st):
                """Stats + output matmuls for block b. Emitted after block
                b+1's layer-0 matmuls; the tanh/rsqrt finalize happens on the
                host."""
                yp = ps_aux.tile([4, BLK], FP32, tag="aux", name="yp")
                for kc in range(4):
                    nc.tensor.matmul(
                        yp, wout_sb[:, kc, :], st["x7"][:, kc, :],
                        start=(kc == 0), stop=(kc == 3),
                    )
                gp7 = ps_aux.tile([4, BLK], FP32, tag="aux", name="gp7")
                nc.tensor.matmul(gp7, ones4_sb, st["sq7"][:, 0, :],
                                 start=True, stop=True)
                # drain raw results to SBUF (frees the PSUM banks without
                # making the PE wait on the DMA queue)
                oyp = scr.tile([4, BLK], FP32, tag="oyp", bufs=2)
                nc.vector.tensor_copy(oyp, yp)
                og7 = scr.tile([1, BLK], FP32, tag="og7", bufs=2)
                nc.vector.tensor_copy(og7, gp7[0:1, :])
                nc.sync.dma_start(out=oraw_d[0:4, b * BLK:(b + 1) * BLK], in_=oyp)
                nc.sync.dma_start(out=oraw_d[4:5, b * BLK:(b + 1) * BLK], in_=og7)

            # ---------------- main loop ----------------
            h = preamble_a0(0)
            for g in preamble_a1_groups(0, h):
                g()
            preamble_b(0, h)
            epi_st = None

            for b in range(n_blocks):
                cur = h
                emit_ff(b, cur)
                psums = layer0(b, cur)
                x_cur, _ = relu_layer(b, 0, psums, want_sq=False)
                if epi_st is not None:
                    epilogue(b - 1, epi_st)

                if b + 2 < n_blocks:
                    inb0_tiles[b + 2] = dma_inb(b + 2)

                sq7 = None
                tp_groups = None
                for j in range(1, N_LAYERS):
                    psums = hidden_mms(b, j, x_cur,
                                       interleave=tp_groups if j == 3 else None)
                    x_cur, sq = relu_layer(b, j, psums, want_sq=(j == 7))
                    if j == 7:
                        sq7 = sq
                    # pipeline next block's preamble into this block's stream
                    if b + 1 < n_blocks:
                        if j == 1:
                            h = preamble_a0(b + 1)
                        elif j == 2:
                            tp_groups = preamble_a1_groups(b + 1, h)
                        elif j == 3:
                            preamble_b(b + 1, h)

                epi_st = {"sq7": sq7, "x7": x_cur}

            epilogue(n_blocks - 1, epi_st)

    nc.compile()
    return nc


def _unpermute(outT):
    """outT: [C, 8192] with column c of block b holding sample p*4+sc where
    c = sc*128 + p. Return [8192, C] in natural sample order."""
    C = outT.shape[0]
    o = outT.reshape(C, N_BLOCKS, 4, 128).transpose(0, 1, 3, 2).reshape(C, S_CORE)
    return o.T


def kernel(**inputs):
    if _general_case_needed(inputs):
        return _numpy_fallback(inputs)

    from concourse.bass_utils import run_bass_kernel_spmd

    pre = _precompute(inputs)
    inp = np.ascontiguousarray(np.asarray(inputs["input"], np.float32))

    if "nc" not in _NC_CACHE:
        _NC_CACHE["nc"] = _build_bass()
    nc = _NC_CACHE["nc"]

    in_maps = [
        {
            "inp": np.ascontiguousarray(inp[c * S_CORE:(c + 1) * S_CORE]),
            "w0p": pre["w0p"], "whp": pre["whp"],
            "sqs": pre["sqs"], "ones4": pre["ones4"],
            "identb": pre["identb"], "identr": pre["identr"],
            "gaussT": pre["gaussT"], "sel8": pre["sel8"], "woutp": pre["woutp"],
        }
        for c in range(N_CORES)
    ]

    res = run_bass_kernel_spmd(
        nc, in_maps, core_ids=list(range(N_CORES)),
        trace=bool(int(os.environ.get("KERNEL_TRACE", "0"))),
    )
    kernel.last_results = res
    outs = []
    for c in range(N_CORES):
        oraw = _unpermute(np.asarray(res.results[c]["oraw"], np.float64))  # [S,5]
        yp, g7 = oraw[:, 0:4], oraw[:, 4:5]
        r = 1.0 / np.sqrt(g7)
        y = yp * r
        out = np.concatenate([np.tanh(y[:, :1]), 255.0 * y[:, 1:]], axis=1)
        outs.append(out)
    return np.ascontiguousarray(np.concatenate(outs, axis=0).astype(np.float32))


# revision 17
# speedup vs baseline: 1.3053x; 1.3053x over previous
"""Trainium2 Bass kernel for nn_Decoder (latent-grid decoder MLP).

Contract: kernel(**inputs) takes the FULL unsharded inputs (as produced by
setup_inputs()) and returns the FULL [65536, 4] float32 output. Internally the
65536 points are sharded across 8 NeuronCores (pure data parallel); the small
weights are replicated.

Algorithm (mathematically equivalent to the reference):
  - G=2 trilinear interp of a per-sample 2x2x2 grid always lands in cell
    (0,0,0) (indices clip to [0, G-2] = [0,0]), so
    lat_i = sum_m w_m(xyz) * (lat @ A_m), A_m = convT_w[:, :, di, dj, dk].
  - The interp + Fourier features + first MLP layer fold into one matmul:
    u = [w_0*lat, ..., w_7*lat, sin(2 pi ang), cos(2 pi ang)]  (2304 dims),
    h0 = u @ M0 with M0 = [A_stack @ W0_top; W0_sin; W0_cos] (host-folded).
  - LayerNorm mean-subtraction folds into the weights (column centering);
    ln gamma folds in too. The per-sample rstd is deferred via LN's positive
    scale invariance: activations stay unnormalized, and gi2 (squared inverse
    scale) follows gi2' = ssq_w/512 + eps*gi2, applied once at the end.
    Requires all biases and ln_b == 0 (true for this model; a numpy fallback
    covers the general case).
  - ssq_w and the eps*gi2 term are accumulated by the TensorEngine itself
    (weighted-ones stationary operands producing a broadcast PSUM tile).

Performance structure (v2):
  - Input arranged so each partition reads a contiguous 4144B DRAM segment
    per block DMA (samples mapped s = b*512 + p*4 + sc); the resulting
    within-block sample permutation is undone on the host.
  - Stationary weights in bf16 (FWL-eligible fast weight loads, half DMA);
    moving activations stay fp32r (full PE rate at 512-wide).
  - lat transposed via bf16 PE transposes (1 cyc/row vs 2 for fp32).
  - Software-pipelined emission: block b+1's preamble (input gate, PE
    transposes, corner weights, fourier prep, u-chunk builds) is emitted in
    the middle of block b's hidden layers; block b's epilogue (LN-stats
    matmuls, output layer, finalize) is deferred until after block b+1's
    layer-0 matmuls, so the in-order PE queue never stalls on the ACT-engine
    square/relu chain.
  - PSUM: 6 banks round-robin for the layer accumulators (breaks the
    layer-boundary wait on ACT), 2 banks for all small/aux tiles.
"""

import os
import numpy as np

N_CORES = 8
N_TOTAL = 65536
S_CORE = N_TOTAL // N_CORES          # 8192 samples per core
BLK = 512                            # samples per block
N_BLOCKS = S_CORE // BLK             # 16
EPS = 1e-5
N_LAYERS = 8                         # LN+relu layers (layer0 + 7 hidden)


def _bf16(a):
    import ml_dtypes
    return np.ascontiguousarray(np.asarray(a, np.float32).astype(ml_dtypes.bfloat16))


def _precompute(inputs):
    """Host-side weight folding. Returns dict of constant arrays."""
    convT_w = np.asarray(inputs["convT_w"], np.float64)
    W0 = np.asarray(inputs["W0"], np.float64)
    Wh = np.asarray(inputs["Wh"], np.float64)
    ln_g = np.asarray(inputs["ln_g"], np.float64)
    gauss = np.asarray(inputs["gauss"], np.float32)
    W_out = np.asarray(inputs["W_out"], np.float64)

    # A_stack[m*256+i, c] = convT_w[i, c, di, dj, dk], m = 4*di + 2*dj + dk
    A_stack = convT_w.transpose(2, 3, 4, 0, 1).reshape(8 * 256, 512)
    M0 = np.concatenate([A_stack @ W0[:512], W0[512:640], W0[640:768]], axis=0)

    def center_scale(W, g):
        Wc = W - W.mean(axis=1, keepdims=True)
        return Wc * g[None, :]

    W_eff = [center_scale(M0, ln_g[0])] + [
        center_scale(Wh[l], ln_g[l + 1]) for l in range(7)
    ]
    # pack each layer's weights as [128, n_kchunks, 512]
    def pack(W):
        K = W.shape[0]
        kc = K // 128
        return W.reshape(kc, 128, 512).transpose(1, 0, 2).reshape(128, kc * 512)

    w0p = _bf16(pack(W_eff[0]))                                      # [128, 18*512]
    whp = _bf16(np.concatenate([pack(W) for W in W_eff[1:]], axis=1))  # [128, 28*512]
    # LN-stat scale: sq_k = (s_k * h_k)^2 with s_k = 1/(sqrt(512)*g_7[k]),
    # summed by a ones-matmul. (The eps*gi2_6 correction term is ~1e-4
    # relative and is dropped.)
    s7 = (1.0 / (np.sqrt(512.0) * np.abs(ln_g[7]))).astype(np.float32)
    sqs = np.ascontiguousarray(s7.reshape(4, 128).T.copy())          # [128, 4]
    ones4 = np.ones((128, 4), np.float32)

    return {
        "w0p": w0p,
        "whp": whp,
        "sqs": sqs,
        "ones4": ones4,
        "identb": _bf16(np.eye(128, dtype=np.float32)),
        "identr": np.ascontiguousarray(np.eye(128, dtype=np.float32)),
        "gaussT": np.ascontiguousarray(np.concatenate(
            [gauss.T.astype(np.float32), np.zeros((8, 128), np.float32)], axis=0)),
        "sel8": np.ascontiguousarray(np.concatenate(
            [np.zeros((3, 8 * 128), np.float32),
             np.kron(np.eye(8, dtype=np.float32), np.ones((1, 128), np.float32))],
            axis=0)),
        "woutp": _bf16(
            np.asarray(W_out, np.float32).reshape(4, 128, 4)
            .transpose(1, 0, 2).reshape(128, 16)),
    }


def _general_case_needed(inputs):
    z = lambda a: bool(np.all(np.asarray(a) == 0))
    return not (
        z(inputs["convT_b"]) and z(inputs["b0"]) and z(inputs["bh"])
        and z(inputs["ln_b"]) and z(inputs["b_out"])
        and bool(np.all(np.abs(np.asarray(inputs["ln_g"])) > 1e-3))
    )


def _numpy_fallback(inputs):
    """Reference in numpy (slow; only for inputs outside the fast path)."""
    inp = np.asarray(inputs["input"], np.float32)
    convT_w = np.asarray(inputs["convT_w"], np.float32)
    convT_b = np.asarray(inputs["convT_b"], np.float32)
    gauss = np.asarray(inputs["gauss"], np.float32)
    W0 = np.asarray(inputs["W0"], np.float32)
    b0 = np.asarray(inputs["b0"], np.float32)
    Wh = np.asarray(inputs["Wh"], np.float32)
    bh = np.asarray(inputs["bh"], np.float32)
    ln_g = np.asarray(inputs["ln_g"], np.float32)
    ln_b = np.asarray(inputs["ln_b"], np.float32)
    W_out = np.asarray(inputs["W_out"], np.float32)
    b_out = np.asarray(inputs["b_out"], np.float32)
    xyz = inp[:, -3:]
    lat = inp[:, :-3]
    f = (xyz + 1.0) * 0.5
    frac = f - np.clip(f.astype(np.int32), 0, 0)
    A = convT_w.transpose(2, 3, 4, 0, 1)
    lat_i = np.zeros((inp.shape[0], 512), np.float32)
    wx = [1 - frac[:, 0], frac[:, 0]]
    wy = [1 - frac[:, 1], frac[:, 1]]
    wz = [1 - frac[:, 2], frac[:, 2]]
    for di in (0, 1):
        for dj in (0, 1):
            for dk in (0, 1):
                w = (wx[di] * wy[dj] * wz[dk]).astype(np.float32)
                lat_i += (lat @ A[di, dj, dk]) * w[:, None]
    lat_i += convT_b[None, :]
    ang = 2 * np.pi * (xyz @ gauss.T)
    x = np.concatenate([lat_i, np.sin(ang), np.cos(ang)], axis=1)

    def ln(t, g, b):
        mu = t.mean(-1, keepdims=True)
        var = ((t - mu) ** 2).mean(-1, keepdims=True)
        return (t - mu) / np.sqrt(var + EPS) * g + b

    x = np.maximum(ln(x @ W0 + b0, ln_g[0], ln_b[0]), 0)
    for l in range(7):
        x = np.maximum(ln(x @ Wh[l] + bh[l], ln_g[l + 1], ln_b[l + 1]), 0)
    y = x @ W_out + b_out
    return np.concatenate([np.tanh(y[:, :1]), y[:, 1:] * 255.0], axis=1).astype(np.float32)


_NC_CACHE = {}


def _build_bass(s_core=S_CORE):
    """Build the per-core Bass module (SPMD; same program on all 8 cores)."""
    import concourse.bass as bass
    import concourse.bacc as bacc
    import concourse.tile as tile
    from concourse import mybir

    FP32 = mybir.dt.float32
    FP32R = mybir.dt.float32r
    BF16 = mybir.dt.bfloat16
    AF = mybir.ActivationFunctionType
    ALU = mybir.AluOpType
    TWO_PI = float(2.0 * np.pi)
    MAGIC = 12582912.0            # 1.5 * 2^23: fp32 add/sub rounds to integer
    n_blocks = s_core // BLK

    nc = bacc.Bacc("TRN2", target_bir_lowering=False, debug=False)

    inp_d = nc.dram_tensor("inp", [s_core, 259], FP32, kind="ExternalInput").ap()
    w0p_d = nc.dram_tensor("w0p", [128, 18 * 512], BF16, kind="ExternalInput").ap()
    whp_d = nc.dram_tensor("whp", [128, 28 * 512], BF16, kind="ExternalInput").ap()
    sqs_d = nc.dram_tensor("sqs", [128, 4], FP32, kind="ExternalInput").ap()
    ones4_d = nc.dram_tensor("ones4", [128, 4], FP32R, kind="ExternalInput").ap()
    identb_d = nc.dram_tensor("identb", [128, 128], BF16, kind="ExternalInput").ap()
    identr_d = nc.dram_tensor("identr", [128, 128], FP32R, kind="ExternalInput").ap()
    gaussT_d = nc.dram_tensor("gaussT", [11, 128], FP32R, kind="ExternalInput").ap()
    sel8_d = nc.dram_tensor("sel8", [11, 8 * 128], FP32R, kind="ExternalInput").ap()
    woutp_d = nc.dram_tensor("woutp", [128, 16], BF16, kind="ExternalInput").ap()
    oraw_d = nc.dram_tensor("oraw", [5, s_core], FP32, kind="ExternalOutput").ap()

    with tile.TileContext(nc) as tc:
        with (
            tc.tile_pool(name="const", bufs=1) as constp,
            tc.tile_pool(name="weights", bufs=1) as weightp,
            tc.tile_pool(name="inblk", bufs=2) as inp_pool,
            tc.tile_pool(name="acts", bufs=2) as actp,
            tc.tile_pool(name="scratch", bufs=2) as scr,
            tc.tile_pool(name="ps_main", bufs=5, space="PSUM") as ps_main,
            tc.tile_pool(name="ps_aux", bufs=2, space="PSUM") as ps_aux,
        ):
            # ---- constants / weights (loaded once, resident) ----
            # Input block DMAs are issued first (below) so block 0's preamble
            # is not stuck behind the weight DMAs on the sync queue.
            # sample s = b*512 + p*4 + sc  ->  [b][p][sc][f], 4144B contiguous
            # per partition per block.
            inp_r = inp_d.rearrange("(b p sc) f -> b p sc f", p=128, sc=4)

            def dma_inb(b):
                t = inp_pool.tile([128, 4, 259], FP32, tag="inb0", name="inb0")
                nc.sync.dma_start(out=t, in_=inp_r[b])
                return t

            inb0_tiles = {}
            inb0_tiles[0] = dma_inb(0)
            inb0_tiles[1] = dma_inb(1)

            identb_dma = constp.tile([128, 128], BF16, name="identb_dma")
            nc.sync.dma_start(out=identb_dma, in_=identb_d)
            identb_sb = constp.tile([128, 128], BF16, name="identb_sb")
            nc.vector.tensor_copy(identb_sb, identb_dma)
            identr_dma = constp.tile([128, 128], FP32R, name="identr_dma")
            nc.sync.dma_start(out=identr_dma, in_=identr_d)
            identr_sb = constp.tile([128, 128], FP32R, name="identr_sb")
            nc.vector.tensor_copy(identr_sb, identr_dma)
            gaussT_sb = constp.tile([11, 128], FP32R)
            nc.sync.dma_start(out=gaussT_sb, in_=gaussT_d)
            sel8_sb = constp.tile([11, 8, 128], FP32R)
            nc.sync.dma_start(out=sel8_sb, in_=sel8_d.rearrange("p (m f) -> p m f", m=8))
            wout_sb = weightp.tile([128, 4, 4], BF16)
            nc.sync.dma_start(out=wout_sb, in_=woutp_d.rearrange("p (c f) -> p c f", c=4))
            sqs_sb = weightp.tile([128, 4], FP32)
            nc.sync.dma_start(out=sqs_sb, in_=sqs_d)
            ones4_sb = weightp.tile([128, 4], FP32R)
            nc.sync.dma_start(out=ones4_sb, in_=ones4_d)

            w0_sb = weightp.tile([128, 18, 512], BF16)
            w0r = w0p_d.rearrange("p (c f) -> p c f", c=18)
            for ch in range(3):
                nc.sync.dma_start(
                    out=w0_sb[:, ch * 6:(ch + 1) * 6, :], in_=w0r[:, ch * 6:(ch + 1) * 6, :])
            wh_sb = weightp.tile([128, 28, 512], BF16)
            whr = whp_d.rearrange("p (c f) -> p c f", c=28)
            for ch in range(4):
                nc.sync.dma_start(
                    out=wh_sb[:, ch * 7:(ch + 1) * 7, :], in_=whr[:, ch * 7:(ch + 1) * 7, :])

            # ---------------- pipelined block structure ----------------

            def preamble_a0(b):
                """Input gate + corner-weight build (DVE only) for block b.
                Emitted during block b-1's layer 1."""
                h = {}
                inb0 = inb0_tiles.pop(b)
                # gate: lat -> bf16 (feeds bf16 PE transposes); xyz -> comb
                latb = inp_pool.tile([128, 4, 256], BF16, tag="latb", name="latb",
                                     bufs=1)
                for sc in range(4):
                    nc.vector.tensor_copy(latb[:, sc, :], inb0[:, sc, 0:256])
                # comb[:, :, 0:3] = xyz, [:, :, 3:11] = corner weights w8
                comb = scr.tile([128, 4, 11], FP32R, tag="comb")
                nc.vector.tensor_copy(comb[:, :, 0:3], inb0[:, :, 256:259])
                xyzf = comb[:, :, 0:3]

                # corner-weight chain (sample-major)
                f3 = scr.tile([128, 4, 3], FP32R, tag="f3")
                nc.vector.tensor_scalar(
                    out=f3, in0=xyzf, scalar1=0.5, scalar2=0.5,
                    op0=ALU.mult, op1=ALU.add,
                )
                om3 = scr.tile([128, 4, 3], FP32R, tag="om3")
                nc.vector.tensor_scalar(
                    out=om3, in0=f3, scalar1=1.0, scalar2=-1.0,
                    op0=ALU.subtract, op1=ALU.mult,
                )
                wxy = scr.tile([128, 4, 4], FP32R, tag="wxy")
                for di in (0, 1):
                    xs = (f3 if di else om3)[:, :, 0:1]
                    for dj in (0, 1):
                        ys = (f3 if dj else om3)[:, :, 1:2]
                        nc.vector.tensor_tensor(
                            out=wxy[:, :, di * 2 + dj:di * 2 + dj + 1],
                            in0=xs, in1=ys, op=ALU.mult,
                        )
                for m in range(8):
                    di, dj, dk = (m >> 2) & 1, (m >> 1) & 1, m & 1
                    zsl = (f3 if dk else om3)[:, :, 2:3]
                    nc.vector.tensor_tensor(
                        out=comb[:, :, 3 + m:4 + m],
                        in0=wxy[:, :, di * 2 + dj:di * 2 + dj + 1],
                        in1=zsl, op=ALU.mult,
                    )

                h.update(latb=latb, comb=comb)
                return h

            def preamble_a1_groups(b, h):
                """PE transpose groups for block b, returned as closures that
                get interleaved between hidden-layer matmul groups (so the
                PSUM-drain copies never stall the PE). 4 transposes share one
                PSUM tile; one merged drain copy each."""
                latb, comb = h["latb"], h["comb"]
                latT = scr.tile([128, 2, BLK], BF16, tag="latT", bufs=1)
                combT = scr.tile([11, BLK], FP32R, tag="combT", bufs=1)

                def lat_half(half):
                    def go():
                        tpl = ps_aux.tile([128, 4, 128], BF16, tag="tp", bufs=1,
                                          name="tpl")
                        for s2 in range(2):
                            sc = half * 2 + s2
                            for fc in range(2):
                                nc.tensor.transpose(
                                    tpl[:, s2 * 2 + fc, :],
                                    latb[:, sc, fc * 128:(fc + 1) * 128],
                                    identb_sb
                                )
                        dst = latT[:, :, half * 256:(half + 1) * 256].rearrange(
                            "p f (s q) -> p s f q", s=2)
                        nc.vector.tensor_copy(dst, tpl)
                    return go

                def comb_grp():
                    tpc = ps_aux.tile([11, 4, 128], FP32R, tag="tp", bufs=1,
                                      name="tpc")
                    for sc in range(4):
                        nc.tensor.transpose(tpc[:, sc, :], comb[:, sc, :],
                                            identr_sb)
                    nc.vector.tensor_copy(
                        combT.rearrange("p (s q) -> p s q", s=4), tpc)

                h.update(latT=latT, combT=combT)
                return [lat_half(0), comb_grp, lat_half(1)]

            def preamble_b(b, h):
                """Fourier prep + corner broadcasts + u-chunk builds for block
                b. Emitted mid block b-1; the sin/cos ACT ops are emitted later
                (emit_ff) so they don't delay relu ops on the ACT queue."""
                # fourier angle, range-reduced to [-0.5, 0.5]
                angp = ps_aux.tile([128, BLK], FP32, tag="aux")
                nc.tensor.matmul(angp, gaussT_sb, h["combT"], start=True, stop=True)
                ang_sb = scr.tile([128, BLK], FP32, tag="rr", bufs=2, name="ang_sb")
                nc.vector.tensor_copy(ang_sb, angp)
                zs_r = scr.tile([128, BLK], FP32, tag="rr", bufs=2, name="zs_r")
                nc.vector.tensor_scalar(
                    out=zs_r, in0=ang_sb, scalar1=MAGIC, scalar2=MAGIC,
                    op0=ALU.add, op1=ALU.subtract,
                )
                zs = scr.tile([128, BLK], FP32, tag="zs", bufs=1)
                nc.vector.tensor_sub(zs, ang_sb, zs_r)
                a25 = scr.tile([128, BLK], FP32, tag="a25", bufs=1)
                nc.vector.tensor_scalar_add(out=a25, in0=ang_sb, scalar1=0.25)
                zc_r = scr.tile([128, BLK], FP32, tag="rr", bufs=2, name="zc_r")
                nc.vector.tensor_scalar(
                    out=zc_r, in0=a25, scalar1=MAGIC, scalar2=MAGIC,
                    op0=ALU.add, op1=ALU.subtract,
                )
                zc = scr.tile([128, BLK], FP32, tag="zc", bufs=1)
                nc.vector.tensor_sub(zc, a25, zc_r)
                h.update(zs=zs, zc=zc)

                # corner broadcasts + weighted-lat u chunks
                uchs = []
                for m in range(8):
                    bc = ps_aux.tile([128, BLK], FP32, tag="aux")
                    nc.tensor.matmul(
                        bc, sel8_sb[:, m, :], h["combT"], start=True, stop=True
                    )
                    for kc in range(2):
                        uch = scr.tile([128, BLK], BF16, tag="uch", bufs=18)
                        nc.vector.tensor_tensor(
                            out=uch, in0=h["latT"][:, kc, :], in1=bc, op=ALU.mult
                        )
                        uchs.append(uch)
                h["uchs"] = uchs

            def emit_ff(b, h):
                """sin/cos ACT ops -> u chunks 16/17. Emitted at the start of
                block b's own iteration: the Sin table load and both sins run
                on ACT while the PE streams layer 0 (no relu is urgent)."""
                ffs = scr.tile([128, BLK], BF16, tag="uch", bufs=18)
                nc.scalar.activation(out=ffs, in_=h["zs"], func=AF.Sin, scale=TWO_PI)
                h["uchs"].append(ffs)
                ffc = scr.tile([128, BLK], BF16, tag="uch", bufs=18)
                nc.scalar.activation(out=ffc, in_=h["zc"], func=AF.Sin, scale=TWO_PI)
                h["uchs"].append(ffc)

            def layer0(b, h):
                """18-chunk folded first layer for block b."""
                psums = [ps_main.tile([128, BLK], FP32, tag="pt", name=f"pt{i}") for i in range(4)]
                uchs = h["uchs"]
                for mc in range(4):
                    for i, uch in enumerate(uchs):
                        nc.tensor.matmul(
                            psums[mc],
                            w0_sb[:, i, mc * 128:(mc + 1) * 128],
                            uch,
                            start=(i == 0), stop=(i == 17),
                        )
                return psums

            def relu_layer(b, j, psums, want_sq):
                x_next = actp.tile([128, 4, BLK], BF16, tag="xn")
                for mc in range(4):
                    nc.scalar.activation(
                        out=x_next[:, mc, :], in_=psums[mc], func=AF.Relu
                    )
                sq = None
                if want_sq:
                    # scaled squares: (s_k h_k)^2; chunk-sum on DVE so the
                    # stats need only one ones-matmul
                    sq = scr.tile([128, 4, BLK], FP32R, tag="sq", bufs=2)
                    for mc in range(4):
                        nc.scalar.activation(
                            out=sq[:, mc, :], in_=psums[mc], func=AF.Square,
                            scale=sqs_sb[:, mc:mc + 1],
                        )
                    nc.vector.tensor_tensor(
                        out=sq[:, 0, :], in0=sq[:, 0, :], in1=sq[:, 1, :],
                        op=ALU.add)
                    nc.vector.tensor_tensor(
                        out=sq[:, 2, :], in0=sq[:, 2, :], in1=sq[:, 3, :],
                        op=ALU.add)
                    nc.vector.tensor_tensor(
                        out=sq[:, 0, :], in0=sq[:, 0, :], in1=sq[:, 2, :],
                        op=ALU.add)
                return x_next, sq

            def hidden_mms(b, j, x_cur, interleave=None):
                psums = [ps_main.tile([128, BLK], FP32, tag="pt", name=f"pt{i}") for i in range(4)]
                for mc in range(4):
                    for kc in range(4):
                        nc.tensor.matmul(
                            psums[mc],
                            wh_sb[:, (j - 1) * 4 + kc, mc * 128:(mc + 1) * 128],
                            x_cur[:, kc, :],
                            start=(kc == 0), stop=(kc == 3),
                        )
                    if interleave and mc < len(interleave):
                        interleave[mc]()
                return psums

            def epilogue(b, st):
                """Stats + output matmuls for block b. Emitted after block
                b+1's layer-0 matmuls; the tanh/rsqrt finalize happens on the
                host."""
                yp = ps_aux.tile([4, BLK], FP32, tag="aux", name="yp")
                for kc in range(4):
                    nc.tensor.matmul(
                        yp, wout_sb[:, kc, :], st["x7"][:, kc, :],
                        start=(kc == 0), stop=(kc == 3),
                    )
                gp7 = ps_aux.tile([4, BLK], FP32, tag="aux", name="gp7")
                nc.tensor.matmul(gp7, ones4_sb, st["sq7"][:, 0, :],
                                 start=True, stop=True)
                # drain raw results to SBUF (frees the PSUM banks without
                # making the PE wait on the DMA queue)
                oyp = scr.tile([4, BLK], FP32, tag="oyp", bufs=2)
                nc.vector.tensor_copy(oyp, yp)
                og7 = scr.tile([1, BLK], FP32, tag="og7", bufs=2)
                nc.vector.tensor_copy(og7, gp7[0:1, :])
                nc.sync.dma_start(out=oraw_d[0:4, b * BLK:(b + 1) * BLK], in_=oyp)
                nc.sync.dma_start(out=oraw_d[4:5, b * BLK:(b + 1) * BLK], in_=og7)

            # ---------------- main loop ----------------
            h = preamble_a0(0)
            for g in preamble_a1_groups(0, h):
                g()
            preamble_b(0, h)
            epi_st = None

            for b in range(n_blocks):
                cur = h
                emit_ff(b, cur)
                psums = layer0(b, cur)
                x_cur, _ = relu_layer(b, 0, psums, want_sq=False)
                if epi_st is not None:
                    epilogue(b - 1, epi_st)

                if b + 2 < n_blocks:
                    inb0_tiles[b + 2] = dma_inb(b + 2)

                sq7 = None
                tp_groups = None
                for j in range(1, N_LAYERS):
                    psums = hidden_mms(b, j, x_cur,
                                       interleave=tp_groups if j == 3 else None)
                    x_cur, sq = relu_layer(b, j, psums, want_sq=(j == 7))
                    if j == 7:
                        sq7 = sq
                    # pipeline next block's preamble into this block's stream
                    if b + 1 < n_blocks:
                        if j == 1:
                            h = preamble_a0(b + 1)
                        elif j == 2:
                            tp_groups = preamble_a1_groups(b + 1, h)
                        elif j == 3:
                            preamble_b(b + 1, h)

                epi_st = {"sq7": sq7, "x7": x_cur}

            epilogue(n_blocks - 1, epi_st)

    nc.compile()
    return nc


def _unpermute(outT):
    """outT: [C, 8192] with column c of block b holding sample p*4+sc where
    c = sc*128 + p. Return [8192, C] in natural sample order."""
    C = outT.shape[0]
    o = outT.reshape(C, N_BLOCKS, 4, 128).transpose(0, 1, 3, 2).reshape(C, S_CORE)
    return o.T


def kernel(**inputs):
    if _general_case_needed(inputs):
        return _numpy_fallback(inputs)

    from concourse.bass_utils import run_bass_kernel_spmd

    pre = _precompute(inputs)
    inp = np.ascontiguousarray(np.asarray(inputs["input"], np.float32))

    if "nc" not in _NC_CACHE:
        _NC_CACHE["nc"] = _build_bass()
    nc = _NC_CACHE["nc"]

    in_maps = [
        {
            "inp": np.ascontiguousarray(inp[c * S_CORE:(c + 1) * S_CORE]),
            "w0p": pre["w0p"], "whp": pre["whp"],
            "sqs": pre["sqs"], "ones4": pre["ones4"],
            "identb": pre["identb"], "identr": pre["identr"],
            "gaussT": pre["gaussT"], "sel8": pre["sel8"], "woutp": pre["woutp"],
        }
        for c in range(N_CORES)
    ]

    res = run_bass_kernel_spmd(
        nc, in_maps, core_ids=list(range(N_CORES)),
        trace=bool(int(os.environ.get("KERNEL_TRACE", "0"))),
    )
    kernel.last_results = res
    outs = []
    for c in range(N_CORES):
        oraw = _unpermute(np.asarray(res.results[c]["oraw"], np.float64))  # [S,5]
        yp, g7 = oraw[:, 0:4], oraw[:, 4:5]
        r = 1.0 / np.sqrt(g7)
        y = yp * r
        out = np.concatenate([np.tanh(y[:, :1]), 255.0 * y[:, 1:]], axis=1)
        outs.append(out)
    return np.ascontiguousarray(np.concatenate(outs, axis=0).astype(np.float32))
